# revision 27
# baseline (speedup 1.0000x reference)
"""Trainium2 Bass kernel for nn_DHSpace_22247930593796 (HGT-style GNN message
passing layer). Self-contained: host-side sharding/preprocessing + Bass/Tile
device kernel + unshard.

Strategy (edge-parallel, zero collectives):
  - Edges sorted by target node; consecutive nodes packed into "segments"
    (<=128 nodes, 16 edge-chunk slots of 128 edges each, slots statically
    assigned to the 8 (src_type, relation) combos, 2 slots per combo).
  - Host pre-gathers x[src]^T per chunk slot (the sharding hint's "shard
    edges and their gathered src rows") so all device DMA is sequential.
  - Per chunk, on device: kv = x_src @ [Wk[t] | Wv[t]@BDmsg[r]] (one matmul,
    lhsT = streamed x^T), attention logits against per-segment q_rel banks
    (relation folded into the q projection), exp, and a one-hot scatter
    matmul accumulating numerator U^T and softmax denominators in PSUM.
  - Segment epilogue: normalize, typed update matmul, residual, layernorm.
  - Segments dealt across 8 NeuronCores; each core runs the same program
    (SPMD) on its own streams. Output rows reassembled on host.
"""
import os
import sys
import types

import numpy as np

# ---------------------------------------------------------------------------
# environment shims (axon PJRT path in this container)
# ---------------------------------------------------------------------------
if "/opt/trn_rl_repo" not in sys.path:
    sys.path.insert(0, "/opt/trn_rl_repo")

def _install_shims():
    # NTFF profile hook that boot() skipped (antenv.axon_hooks missing)
    if "antenv.axon_hooks" not in sys.modules:
        mod = types.ModuleType("antenv.axon_hooks")
        _h = [None]
        mod.set_axon_ntff_profile_hook = lambda h: _h.__setitem__(0, h)
        mod.get_axon_ntff_profile_hook = lambda: _h[0]
        sys.modules["antenv.axon_hooks"] = mod
        try:
            import antenv

            antenv.axon_hooks = mod
        except Exception:
            pass
        try:
            from trn_agent_boot.trn_boot import _ntff_profile_via_ctypes

            mod.set_axon_ntff_profile_hook(
                _ntff_profile_via_ctypes("/opt/axon/libaxon_pjrt.so")
            )
        except Exception:
            pass

    import concourse.bass_utils as bass_utils

    bass_utils.upload_artifacts = lambda tmpdir: tmpdir

    # walrus in this image rejects >1 sync wait per instruction: split the
    # final TileContext drain and every scheduled instruction's waits onto
    # same-engine NoOp carriers.
    import concourse.mybir as mybir
    import concourse.tile as tile
    from concourse.vector_clock import ScopedClock

    if getattr(tile.TileContext, "_wait_split_installed", False):
        return

    def _drain_and_barrier_split(self, tick_clock, wait_clock):
        import bass_rust

        d = self.nc.sync.drain()
        wait_clock.add_sem_waits(d.ins, ScopedClock({None: tick_clock.global_clock}))
        si = d.ins.sync_info
        waits = list(si.on_wait) if si is not None and si.on_wait else []
        if len(waits) > 1:
            si.on_wait = waits[:1]
            for i in range(1, len(waits)):
                d2 = self.nc.sync.drain()
                si2 = d2.ins.sync_info
                if si2 is None:
                    d2.ins.sync_info = bass_rust.SyncInfo(
                        on_wait=waits[i : i + 1], on_update=[]
                    )
                else:
                    si2.on_wait = waits[i : i + 1]
        self.nc.all_engine_barrier()
        popped = self.nc._tile_sem_poison_stack.pop()
        assert popped is self._sem_poison
        self.nc.clear_and_free_semaphores(list(self.sems.allocated().values()))
        self.nc.all_engine_barrier()

    tile.TileContext._drain_and_barrier = _drain_and_barrier_split

    _orig_commit = tile.TileContext._commit_instruction

    def _commit_split_waits(self, inst, lazy_reg_writes: bool = True):
        si = getattr(inst, "sync_info", None)
        if si is not None and si.on_wait and len(si.on_wait) > 1:
            waits = list(si.on_wait)
            engine = inst.engine
            if engine is not None and engine != mybir.EngineType.Unassigned:
                si.on_wait = waits[-1:]
                for w in waits[:-1]:
                    nop = mybir.InstNoOp(
                        name=self.nc.get_next_instruction_name(),
                        engine=engine,
                        sync_info=mybir.SyncInfo(on_wait=[w], on_update=[]),
                        bass_nofuse=True,
                    )
                    _orig_commit(self, nop, lazy_reg_writes)
        return _orig_commit(self, inst, lazy_reg_writes)

    tile.TileContext._commit_instruction = _commit_split_waits
    tile.TileContext._wait_split_installed = True


_install_shims()

import concourse.bass as bass
import concourse.mybir as mybir
import concourse.tile as tile
from concourse.bass_utils import run_bass_kernel_spmd
from concourse.masks import make_identity

f32 = mybir.dt.float32
bf16 = mybir.dt.bfloat16
P = 128
N_CORES = 8
USE_BF16 = False  # bf16 sel/attention path: ~1.1e-3 rel err, 1.25ms; fp32: 5e-6, 1.54ms


# ---------------------------------------------------------------------------
# host-side weight folding
# ---------------------------------------------------------------------------
def _blockdiag(mats):
    """mats: [H, a, b] -> blockdiag [H*a, H*b]"""
    H, a, b = mats.shape
    out = np.zeros((H * a, H * b), dtype=mats.dtype)
    for h in range(H):
        out[h * a : (h + 1) * a, h * b : (h + 1) * b] = mats[h]
    return out


def _fold_weights(qW, qb, kW, kb, vW, vb, rel_att, rel_pri, rel_msg, upd_W, upd_b):
    """Returns host-folded weight banks (see kernel layout)."""
    KN, HID, _ = qW.shape
    KR, H, DK, _ = rel_att.shape
    s = 1.0 / np.sqrt(DK)
    # l[e,h] = (q @ G_r)[h,:] . k[h,:]  with G_r = blockdiag_h(rel_att[r,h].T * pri[r,h]*s)
    Wq_fold = np.zeros((KN, KR, HID, HID), np.float32)
    qb_fold = np.zeros((KN, KR, HID), np.float32)
    Wv_fold = np.zeros((KN, KR, HID, HID), np.float32)
    vb_fold = np.zeros((KN, KR, HID), np.float32)
    for r in range(KR):
        G = _blockdiag(
            np.ascontiguousarray(rel_att[r].transpose(0, 2, 1))
            * (rel_pri[r] * s)[:, None, None]
        )
        Bm = _blockdiag(rel_msg[r])
        for t in range(KN):
            Wq_fold[t, r] = qW[t] @ G
            qb_fold[t, r] = qb[t] @ G
            Wv_fold[t, r] = vW[t] @ Bm
            vb_fold[t, r] = vb[t] @ Bm
    return dict(
        Wq_fold=Wq_fold, qb_fold=qb_fold, Wv_fold=Wv_fold, vb_fold=vb_fold,
        Wk=kW.astype(np.float32), kb=kb.astype(np.float32),
        Wu=upd_W.astype(np.float32), ub=upd_b.astype(np.float32),
    )


# ---------------------------------------------------------------------------
# host-side segmentation + stream building
# ---------------------------------------------------------------------------
def _prep(x, node_type, src, tgt, rel, CR, n_cores):
    """Segment nodes, assign edges to chunk slots, build per-core streams."""
    N, HID = x.shape
    KCOMB = 8
    C = KCOMB * CR
    cap = CR * P

    combo = (node_type[src] * 4 + rel).astype(np.int64)

    # per (node, combo) incoming-edge counts
    cnt = np.bincount(tgt * KCOMB + combo, minlength=N * KCOMB).reshape(N, KCOMB)

    # greedy segmentation over node order
    seg_start = [0]
    cur = np.zeros(KCOMB, np.int64)
    start = 0
    for n in range(N):
        c = cnt[n]
        if (n - start) >= P or np.any(cur + c > cap):
            seg_start.append(n)
            start = n
            cur = c.copy()
        else:
            cur += c
    seg_start = np.asarray(seg_start, np.int64)
    seg_end = np.append(seg_start[1:], N)
    S_total = len(seg_start)

    S = (S_total + n_cores - 1) // n_cores

    # map each node -> (segment, local index)
    node_seg = np.repeat(np.arange(S_total), seg_end - seg_start)
    node_loc = np.arange(N) - seg_start[node_seg]

    # edge -> segment, combo; slot position within (segment, combo)
    e_seg = node_seg[tgt]
    e_key = e_seg * KCOMB + combo
    order = np.argsort(e_key, kind="stable")
    ranks = np.empty(len(order), np.int64)
    keys_sorted = e_key[order]
    # rank within each (segment, combo) group
    grp_first = np.r_[0, np.flatnonzero(np.diff(keys_sorted)) + 1]
    grp_id = np.cumsum(np.r_[0, np.diff(keys_sorted) != 0])
    ranks_sorted = np.arange(len(order)) - grp_first[grp_id]
    ranks[order] = ranks_sorted
    assert ranks.max() < cap, "segment combo capacity overflow"

    e_chunk = combo * CR + ranks // P   # chunk slot 0..C-1 within segment
    e_pos = ranks % P                   # partition row within chunk
    e_core = e_seg // S
    e_sloc = e_seg % S

    # streams
    xT = np.zeros((n_cores, S, HID, C * P), np.float32)
    tgtl = np.full((n_cores, S, P, C), -1.0, np.float32)
    xsegT = np.zeros((n_cores, S, HID, 2 * P), np.float32)
    xseg = np.zeros((n_cores, S, P, HID), np.float32)
    typec = np.zeros((n_cores, S, P, 1), np.float32)
    tmask = np.zeros((n_cores, S, 2, P), np.float32)

    col = e_chunk * P + e_pos
    xT[e_core, e_sloc, :, col] = x[src]
    tgtl[e_core, e_sloc, e_pos, e_chunk] = node_loc[tgt].astype(np.float32)
    selS = np.zeros((n_cores, S, P, C * P), np.float32)
    selS[e_core, e_sloc, e_pos, e_chunk * P + node_loc[tgt]] = 1.0
    selT = np.zeros((n_cores, S, P, C * P), np.float32)
    selT[e_core, e_sloc, node_loc[tgt], e_chunk * P + e_pos] = 1.0

    n_seg = node_seg
    n_core = n_seg // S
    n_sloc = n_seg % S
    xseg[n_core, n_sloc, node_loc, :] = x
    m0 = node_type == 0
    xsegT[n_core[m0], n_sloc[m0], :, node_loc[m0]] = x[m0]
    m1 = ~m0
    xsegT[n_core[m1], n_sloc[m1], :, P + node_loc[m1]] = x[m1]
    typec[n_core, n_sloc, node_loc, 0] = node_type.astype(np.float32)
    tmask[n_core, n_sloc, node_type, node_loc] = 1.0

    unshard = (S, seg_start, seg_end)
    streams = dict(xT=xT, tgtl=tgtl, xsegT=xsegT, xseg=xseg, typec=typec,
                   tmask=tmask, sel=selS, selT=selT)
    return streams, unshard, S, C


# ---------------------------------------------------------------------------
# device program
# ---------------------------------------------------------------------------
def _build_program(S, C, CR, flags):
    """One SPMD program processing S segments of C chunk slots."""
    GROUP = 2
    NG = C // GROUP
    dt_e = bf16 if USE_BF16 else f32  # edge-path matmul operand dtype
    nc = bass.Bass("TRN2", target_bir_lowering=False, debug=False)

    xT_d = nc.declare_dram_parameter("xT", [S, P, C * P], f32, isOutput=False)
    sel_d = nc.declare_dram_parameter("sel", [S, P, C * P], f32, isOutput=False)
    selT_d = nc.declare_dram_parameter("selT", [S, P, C * P], bf16, isOutput=False)
    xsegT_d = nc.declare_dram_parameter("xsegT", [S, P, 2 * P], f32, isOutput=False)
    xseg_d = nc.declare_dram_parameter("xseg", [S, P, P], f32, isOutput=False)
    typec_d = nc.declare_dram_parameter("typec", [S, P, 1], f32, isOutput=False)
    Wkv_d = nc.declare_dram_parameter("Wkv", [P, 8 * 256], f32, isOutput=False)
    Wq_d = nc.declare_dram_parameter("Wq", [P, 8 * P], f32, isOutput=False)
    Wu_d = nc.declare_dram_parameter("Wu", [P, 2 * P], f32, isOutput=False)
    B8T_d = nc.declare_dram_parameter("B8T", [8, P], f32, isOutput=False)
    if flags["kv_bias"]:
        bkv_d = nc.declare_dram_parameter("bkv", [1, 8 * 256], f32, isOutput=False)
    if flags["q_bias"]:
        bq_d = nc.declare_dram_parameter("bq", [1, 8 * P], f32, isOutput=False)
        tmask_d = nc.declare_dram_parameter("tmask", [S, 2, P], f32, isOutput=False)
    if flags["u_bias"]:
        bu_d = nc.declare_dram_parameter("bu", [1, 2 * P], f32, isOutput=False)
    if flags["ln_affine"]:
        lns_d = nc.declare_dram_parameter("lns", [1, 2 * P], f32, isOutput=False)
        lnb_d = nc.declare_dram_parameter("lnb", [1, 2 * P], f32, isOutput=False)
    out_d = nc.declare_dram_parameter("out", [S, P, P], f32, isOutput=True)

    from contextlib import ExitStack

    with tile.TileContext(nc) as tc, ExitStack() as ctx:
        cpool = ctx.enter_context(tc.tile_pool(name="const", bufs=1))
        ones = cpool.tile([1, P], f32)
        nc.gpsimd.memset(ones[:], 1.0)
        eps_t = cpool.tile([P, 1], f32)
        nc.gpsimd.memset(eps_t[:], 1e-5)

        Wkv_t = cpool.tile([P, 8 * 256], f32)
        nc.sync.dma_start(Wkv_t[:], Wkv_d[:])
        Wq_t = cpool.tile([P, 8 * P], f32)
        nc.sync.dma_start(Wq_t[:], Wq_d[:])
        Wu_t = cpool.tile([P, 2 * P], f32)
        nc.sync.dma_start(Wu_t[:], Wu_d[:])
        B8T_t = cpool.tile([8, P], f32)
        nc.sync.dma_start(B8T_t[:], B8T_d[:])
        if flags["kv_bias"]:
            bkv_t = cpool.tile([1, 8 * 256], f32)
            nc.sync.dma_start(bkv_t[:], bkv_d[:])
        if flags["q_bias"]:
            bq_t = cpool.tile([1, 8 * P], f32)
            nc.sync.dma_start(bq_t[:], bq_d[:])
        if flags["u_bias"]:
            bu_t = cpool.tile([1, 2 * P], f32)
            nc.sync.dma_start(bu_t[:], bu_d[:])
        if flags["ln_affine"]:
            lns_t = cpool.tile([P, 2 * P], f32)
            nc.sync.dma_start(lns_t[:], lns_d[:1, :].to_broadcast([P, 2 * P]))
            lnb_t = cpool.tile([P, 2 * P], f32)
            nc.sync.dma_start(lnb_t[:], lnb_d[:1, :].to_broadcast([P, 2 * P]))

        # stream pools
        xT_p = ctx.enter_context(tc.tile_pool(name="xT", bufs=3))
        tg_p = ctx.enter_context(tc.tile_pool(name="tg", bufs=2))
        xsT_p = ctx.enter_context(tc.tile_pool(name="xsT", bufs=2))
        xs_p = ctx.enter_context(tc.tile_pool(name="xs", bufs=2))
        ty_p = ctx.enter_context(tc.tile_pool(name="ty", bufs=2))
        sel_p = ctx.enter_context(tc.tile_pool(name="sel", bufs=3))
        sb_p = ctx.enter_context(tc.tile_pool(name="sb", bufs=4))      # slab SBUF
        node_p = ctx.enter_context(tc.tile_pool(name="node", bufs=2))  # node-phase

        kv_ps = ctx.enter_context(tc.tile_pool(name="kvps", bufs=3, space="PSUM"))
        st_ps = ctx.enter_context(tc.tile_pool(name="stps", bufs=1, space="PSUM"))
        qe_ps = ctx.enter_context(tc.tile_pool(name="qeps", bufs=2, space="PSUM"))
        u_ps = ctx.enter_context(tc.tile_pool(name="ups", bufs=1, space="PSUM"))
        d_ps = ctx.enter_context(tc.tile_pool(name="dps", bufs=1, space="PSUM"))
        n_ps = ctx.enter_context(tc.tile_pool(name="nps", bufs=1, space="PSUM"))

        for s in range(S):
            # ---- loads ----
            xT_t = xT_p.tile([P, C * P], f32)
            nc.sync.dma_start(xT_t[:], xT_d[s])
            sel_t = sel_p.tile([P, C * P], f32)
            nc.sync.dma_start(sel_t[:], sel_d[s])
            selT_t = sel_p.tile([P, C * P], bf16, tag="selT")
            nc.sync.dma_start(selT_t[:], selT_d[s])
            xsT_t = xsT_p.tile([P, 2 * P], f32)
            nc.sync.dma_start(xsT_t[:], xsegT_d[s])
            xs_t = xs_p.tile([P, P], f32)
            nc.sync.dma_start(xs_t[:], xseg_d[s])
            ty_t = ty_p.tile([P, 1], f32)
            nc.sync.dma_start(ty_t[:], typec_d[s])

            # ---- q_rel banks: q folded per relation ----
            qr_ps = n_ps.tile([P, 4 * P], f32, tag="nps")
            for r in range(4):
                nc.tensor.matmul(
                    qr_ps[:, r * P : (r + 1) * P],
                    lhsT=xsT_t[:, 0:P],
                    rhs=Wq_t[:, r * P : (r + 1) * P],
                    start=True, stop=False,
                )
                nc.tensor.matmul(
                    qr_ps[:, r * P : (r + 1) * P],
                    lhsT=xsT_t[:, P : 2 * P],
                    rhs=Wq_t[:, (4 + r) * P : (5 + r) * P],
                    start=False, stop=not flags["q_bias"],
                )
            if flags["q_bias"]:
                # typed bias via rank-1 mms with host-streamed per-type masks
                tm_t = node_p.tile([2, P], f32, tag="tm")
                nc.sync.dma_start(tm_t[:], tmask_d[s])
                for r in range(4):
                    nc.tensor.matmul(
                        qr_ps[:, r * P : (r + 1) * P],
                        lhsT=tm_t[0:1, :], rhs=bq_t[:1, r * P : (r + 1) * P],
                        start=False, stop=False,
                    )
                    nc.tensor.matmul(
                        qr_ps[:, r * P : (r + 1) * P],
                        lhsT=tm_t[1:2, :], rhs=bq_t[:1, (4 + r) * P : (5 + r) * P],
                        start=False, stop=True,
                    )
            qr_sb = sb_p.tile([P, 4 * P], bf16, tag="qr")
            nc.scalar.copy(qr_sb[:], qr_ps[:])
            qr_lo = sb_p.tile([P, 4 * P], bf16, tag="qrlo")
            nc.vector.tensor_tensor(
                out=qr_lo[:], in0=qr_ps[:], in1=qr_sb[:],
                op=mybir.AluOpType.subtract,
            )

            # ---- accumulators (zeroed by the c==0 scatter matmuls) ----
            U_t = u_ps.tile([P, P], f32, tag="U")
            den_t = d_ps.tile([8, P], f32, tag="den")

            # ---- edge chunk groups ----
            for g in range(NG):
                kv_t = kv_ps.tile([P, GROUP * 256], f32, tag="kv")
                for j in range(GROUP):
                    c = g * GROUP + j
                    m = c // CR  # combo = t*4+r
                    nc.tensor.matmul(
                        kv_t[:, j * 256 : (j + 1) * 256],
                        lhsT=xT_t[:, c * P : (c + 1) * P],
                        rhs=Wkv_t[:, m * 256 : (m + 1) * 256],
                        start=True, stop=not flags["kv_bias"],
                    )
                    if flags["kv_bias"]:
                        nc.tensor.matmul(
                            kv_t[:, j * 256 : (j + 1) * 256],
                            lhsT=ones[:1, :],
                            rhs=bkv_t[:1, m * 256 : (m + 1) * 256],
                            start=False, stop=True,
                        )

                qe_t = qe_ps.tile([P, GROUP * P], f32, tag="qe")
                for j in range(GROUP):
                    c = g * GROUP + j
                    r = (c // CR) % 4
                    nc.tensor.matmul(
                        qe_t[:, j * P : (j + 1) * P],
                        lhsT=selT_t[:, c * P : (c + 1) * P],
                        rhs=qr_sb[:, r * P : (r + 1) * P],
                        start=True, stop=False,
                    )
                    nc.tensor.matmul(
                        qe_t[:, j * P : (j + 1) * P],
                        lhsT=selT_t[:, c * P : (c + 1) * P],
                        rhs=qr_lo[:, r * P : (r + 1) * P],
                        start=False, stop=True,
                    )
                qe_sb = sb_p.tile([P, GROUP * P], f32, tag="qes")
                nc.scalar.copy(qe_sb[:], qe_t[:])

                kvv = kv_t[:].rearrange("p (j s) -> p j s", s=256)
                qk_sb = sb_p.tile([P, GROUP * P], f32, tag="qk")
                nc.vector.tensor_tensor(
                    out=qk_sb[:].rearrange("p (j n) -> p j n", j=GROUP),
                    in0=qe_sb[:].rearrange("p (j n) -> p j n", j=GROUP),
                    in1=kvv[:, :, 0:P],
                    op=mybir.AluOpType.mult,
                )
                l_sb = sb_p.tile([P, GROUP * 8], f32, tag="l")
                nc.vector.reduce_sum(
                    out=l_sb[:],
                    in_=qk_sb[:].rearrange("p (a k) -> p a k", k=16),
                    axis=mybir.AxisListType.X,
                )
                w_sb = sb_p.tile([P, GROUP * 8], dt_e, tag="w")
                nc.scalar.activation(w_sb[:], l_sb[:], mybir.ActivationFunctionType.Exp)
                wv_sb = sb_p.tile([P, GROUP * P], dt_e, tag="wv")
                nc.vector.tensor_tensor(
                    out=wv_sb[:].rearrange("p (j h k) -> p j h k", j=GROUP, h=8),
                    in0=kvv[:, :, P:256].rearrange("p j (h k) -> p j h k", h=8),
                    in1=w_sb[:].rearrange("p (j h) -> p j h", j=GROUP)[:, :, :, None]
                    .to_broadcast([P, GROUP, 8, 16]),
                    op=mybir.AluOpType.mult,
                )
                last = (g == NG - 1)
                for j in range(GROUP):
                    c = g * GROUP + j
                    nc.tensor.matmul(
                        U_t[:],
                        lhsT=wv_sb[:, j * P : (j + 1) * P],
                        rhs=sel_t[:, c * P : (c + 1) * P],
                        start=c == 0, stop=last and j == GROUP - 1,
                    )
                    nc.tensor.matmul(
                        den_t[:],
                        lhsT=w_sb[:, j * 8 : (j + 1) * 8],
                        rhs=sel_t[:, c * P : (c + 1) * P],
                        start=c == 0, stop=last and j == GROUP - 1,
                    )

            # ---- segment epilogue ----
            dmax = node_p.tile([8, P], f32, tag="dmax")
            nc.vector.tensor_scalar(
                out=dmax[:], in0=den_t[:], scalar1=1e-9, scalar2=None,
                op0=mybir.AluOpType.max,
            )
            drec = node_p.tile([8, P], f32, tag="drec")
            nc.vector.reciprocal(drec[:], dmax[:])
            dbc_t = n_ps.tile([P, P], f32, tag="nps")
            nc.tensor.matmul(dbc_t[:], lhsT=B8T_t[:], rhs=drec[:], start=True, stop=True)
            dbc_sb = node_p.tile([P, P], f32, tag="dbc")
            nc.scalar.copy(dbc_sb[:], dbc_t[:])
            aggrn = node_p.tile([P, P], f32, tag="aggrn")
            nc.vector.tensor_tensor(
                out=aggrn[:], in0=U_t[:], in1=dbc_sb[:], op=mybir.AluOpType.mult
            )
            upd_t = n_ps.tile([P, 2 * P], f32, tag="nps")
            for t in range(2):
                nc.tensor.matmul(
                    upd_t[:, t * P : (t + 1) * P],
                    lhsT=aggrn[:], rhs=Wu_t[:, t * P : (t + 1) * P],
                    start=True, stop=not flags["u_bias"],
                )
                if flags["u_bias"]:
                    nc.tensor.matmul(
                        upd_t[:, t * P : (t + 1) * P],
                        lhsT=ones[:1, :], rhs=bu_t[:1, t * P : (t + 1) * P],
                        start=False, stop=True,
                    )
            upd_sb = node_p.tile([P, 2 * P], f32, tag="upd")
            nc.scalar.copy(upd_sb[:], upd_t[:])
            diff = node_p.tile([P, P], f32, tag="diff")
            nc.vector.tensor_tensor(
                out=diff[:], in0=upd_sb[:, P : 2 * P], in1=upd_sb[:, 0:P],
                op=mybir.AluOpType.subtract,
            )
            # blended update + residual: upd0 + typec*diff + x
            b1 = node_p.tile([P, P], f32, tag="b1")
            nc.vector.scalar_tensor_tensor(
                out=b1[:], in0=diff[:], scalar=ty_t[:, 0:1], in1=upd_sb[:, 0:P],
                op0=mybir.AluOpType.mult, op1=mybir.AluOpType.add,
            )
            h_t = node_p.tile([P, P], f32, tag="h")
            nc.vector.tensor_tensor(
                out=h_t[:], in0=b1[:], in1=xs_t[:], op=mybir.AluOpType.add
            )
            # layernorm
            musum = node_p.tile([P, 1], f32, tag="musum")
            nc.vector.reduce_sum(out=musum[:], in_=h_t[:], axis=mybir.AxisListType.X)
            mean = node_p.tile([P, 1], f32, tag="mean")
            nc.scalar.mul(mean[:], musum[:], 1.0 / P)
            xc = node_p.tile([P, P], f32, tag="xc")
            nc.vector.tensor_scalar(
                out=xc[:], in0=h_t[:], scalar1=mean[:, 0:1], scalar2=None,
                op0=mybir.AluOpType.subtract,
            )
            sq = node_p.tile([P, P], f32, tag="sq")
            ss = node_p.tile([P, 1], f32, tag="ss")
            nc.scalar.activation(
                sq[:], xc[:], mybir.ActivationFunctionType.Square, accum_out=ss[:]
            )
            sd = node_p.tile([P, 1], f32, tag="sd")
            nc.scalar.activation(
                sd[:], ss[:], mybir.ActivationFunctionType.Sqrt,
                bias=eps_t[:, 0:1], scale=1.0 / P,
            )
            rstd = node_p.tile([P, 1], f32, tag="rstd")
            nc.vector.reciprocal(rstd[:], sd[:])
            o_t = node_p.tile([P, P], f32, tag="o")
            nc.vector.tensor_scalar(
                out=o_t[:], in0=xc[:], scalar1=rstd[:, 0:1], scalar2=None,
                op0=mybir.AluOpType.mult,
            )
            if flags["ln_affine"]:
                sdiff = node_p.tile([P, P], f32, tag="sdiff")
                nc.vector.tensor_tensor(
                    out=sdiff[:], in0=lns_t[:, P : 2 * P], in1=lns_t[:, 0:P],
                    op=mybir.AluOpType.subtract,
                )
                seff = node_p.tile([P, P], f32, tag="seff")
                nc.vector.scalar_tensor_tensor(
                    out=seff[:], in0=sdiff[:], scalar=ty_t[:, 0:1], in1=lns_t[:, 0:P],
                    op0=mybir.AluOpType.mult, op1=mybir.AluOpType.add,
                )
                o2 = node_p.tile([P, P], f32, tag="o2")
                nc.vector.tensor_tensor(
                    out=o2[:], in0=o_t[:], in1=seff[:], op=mybir.AluOpType.mult
                )
                bdiff = node_p.tile([P, P], f32, tag="bdiff")
                nc.vector.tensor_tensor(
                    out=bdiff[:], in0=lnb_t[:, P : 2 * P], in1=lnb_t[:, 0:P],
                    op=mybir.AluOpType.subtract,
                )
                beff = node_p.tile([P, P], f32, tag="beff")
                nc.vector.scalar_tensor_tensor(
                    out=beff[:], in0=bdiff[:], scalar=ty_t[:, 0:1], in1=lnb_t[:, 0:P],
                    op0=mybir.AluOpType.mult, op1=mybir.AluOpType.add,
                )
                o3 = node_p.tile([P, P], f32, tag="o3")
                nc.vector.tensor_tensor(
                    out=o3[:], in0=o2[:], in1=beff[:], op=mybir.AluOpType.add
                )
                o_t = o3
            nc.sync.dma_start(out_d[s], o_t[:])

    return nc


# ---------------------------------------------------------------------------
# entry point
# ---------------------------------------------------------------------------
def kernel(x, node_type, edge_index, edge_rel, qW, qb, kW, kb, vW, vb,
           rel_att, rel_pri, rel_msg, upd_W, upd_b, ln_scale, ln_bias,
           CR=2, trace=False):
    x = np.asarray(x, np.float32)
    node_type = np.asarray(node_type).astype(np.int64)
    edge_index = np.asarray(edge_index).astype(np.int64)
    edge_rel = np.asarray(edge_rel).astype(np.int64)
    qW, qb, kW, kb = (np.asarray(a, np.float32) for a in (qW, qb, kW, kb))
    vW, vb = np.asarray(vW, np.float32), np.asarray(vb, np.float32)
    rel_att = np.asarray(rel_att, np.float32)
    rel_pri = np.asarray(rel_pri, np.float32)
    rel_msg = np.asarray(rel_msg, np.float32)
    upd_W, upd_b = np.asarray(upd_W, np.float32), np.asarray(upd_b, np.float32)
    ln_scale, ln_bias = np.asarray(ln_scale, np.float32), np.asarray(ln_bias, np.float32)

    N, HID = x.shape
    src, tgt = edge_index[0], edge_index[1]

    fw = _fold_weights(qW, qb, kW, kb, vW, vb, rel_att, rel_pri, rel_msg, upd_W, upd_b)

    flags = dict(
        kv_bias=bool(np.any(fw["kb"]) or np.any(fw["vb_fold"])),
        q_bias=bool(np.any(fw["qb_fold"])),
        u_bias=bool(np.any(fw["ub"])),
        ln_affine=bool(np.any(ln_scale != 1.0) or np.any(ln_bias != 0.0)),
    )

    streams, unshard, S, C = _prep(x, node_type, src, tgt, edge_rel, CR, N_CORES)

    # weight banks
    Wkv = np.zeros((P, 8 * 256), np.float32)
    bkv = np.zeros((1, 8 * 256), np.float32)
    Wq = np.zeros((P, 8 * P), np.float32)
    bq = np.zeros((1, 8 * P), np.float32)
    for t in range(2):
        for r in range(4):
            m = t * 4 + r
            Wkv[:, m * 256 : m * 256 + P] = fw["Wk"][t]
            Wkv[:, m * 256 + P : (m + 1) * 256] = fw["Wv_fold"][t, r]
            bkv[0, m * 256 : m * 256 + P] = fw["kb"][t]
            bkv[0, m * 256 + P : (m + 1) * 256] = fw["vb_fold"][t, r]
            Wq[:, m * P : (m + 1) * P] = fw["Wq_fold"][t, r]
            bq[0, m * P : (m + 1) * P] = fw["qb_fold"][t, r]
    Wu = np.concatenate([fw["Wu"][0], fw["Wu"][1]], axis=1)
    bu = np.concatenate([fw["ub"][0], fw["ub"][1]])[None, :]
    B8T = (np.arange(P)[None, :] // 16 == np.arange(8)[:, None]).astype(np.float32)

    nc = _build_program(S, C, CR, flags)

    import ml_dtypes

    streams["selT"] = streams["selT"].astype(ml_dtypes.bfloat16)

    in_maps = []
    for c in range(N_CORES):
        m = {
            "xT": streams["xT"][c],
            "sel": streams["sel"][c], "selT": streams["selT"][c],
            "xsegT": streams["xsegT"][c], "xseg": streams["xseg"][c],
            "typec": streams["typec"][c],
            "Wkv": Wkv, "Wq": Wq, "Wu": Wu, "B8T": B8T,
        }
        if flags["kv_bias"]:
            m["bkv"] = bkv
        if flags["q_bias"]:
            m["bq"] = bq
            m["tmask"] = streams["tmask"][c]
        if flags["u_bias"]:
            m["bu"] = bu
        if flags["ln_affine"]:
            m["lns"] = np.concatenate([ln_scale[0], ln_scale[1]])[None, :]
            m["lnb"] = np.concatenate([ln_bias[0], ln_bias[1]])[None, :]
        in_maps.append(m)

    res = run_bass_kernel_spmd(nc, in_maps, list(range(N_CORES)), trace=trace)

    S_, seg_start, seg_end = unshard
    out = np.empty((N, HID), np.float32)
    for i in range(len(seg_start)):
        c, sl = i // S_, i % S_
        a, b = seg_start[i], seg_end[i]
        out[a:b] = res.results[c]["out"][sl, : b - a]
    kernel.last_exec_time_ns = res.exec_time_ns
    kernel.last_results = res
    return out


kernel.last_exec_time_ns = None
kernel.last_results = None


# revision 28
# speedup vs baseline: 1.3101x; 1.3101x over previous
"""Trainium2 Bass kernel for nn_DHSpace_22247930593796 (HGT-style GNN message
passing layer). Self-contained: host-side sharding/preprocessing + Bass/Tile
device kernel + unshard.

Strategy (edge-parallel, zero collectives):
  - Edges sorted by target node; consecutive nodes packed into "segments"
    (<=128 nodes, 16 edge-chunk slots of 128 edges each, slots statically
    assigned to the 8 (src_type, relation) combos, 2 slots per combo).
  - Host pre-gathers x[src]^T per chunk slot (the sharding hint's "shard
    edges and their gathered src rows") so all device DMA is sequential.
  - Per chunk, on device: kv = x_src @ [Wk[t] | Wv[t]@BDmsg[r]] (one matmul,
    lhsT = streamed x^T), attention logits against per-segment q_rel banks
    (relation folded into the q projection), exp, and a one-hot scatter
    matmul accumulating numerator U^T and softmax denominators in PSUM.
  - Segment epilogue: normalize, typed update matmul, residual, layernorm.
  - Segments dealt across 8 NeuronCores; each core runs the same program
    (SPMD) on its own streams. Output rows reassembled on host.
"""
import os
import sys
import types

import numpy as np

# ---------------------------------------------------------------------------
# environment shims (axon PJRT path in this container)
# ---------------------------------------------------------------------------
if "/opt/trn_rl_repo" not in sys.path:
    sys.path.insert(0, "/opt/trn_rl_repo")

def _install_shims():
    # NTFF profile hook that boot() skipped (antenv.axon_hooks missing)
    if "antenv.axon_hooks" not in sys.modules:
        mod = types.ModuleType("antenv.axon_hooks")
        _h = [None]
        mod.set_axon_ntff_profile_hook = lambda h: _h.__setitem__(0, h)
        mod.get_axon_ntff_profile_hook = lambda: _h[0]
        sys.modules["antenv.axon_hooks"] = mod
        try:
            import antenv

            antenv.axon_hooks = mod
        except Exception:
            pass
        try:
            from trn_agent_boot.trn_boot import _ntff_profile_via_ctypes

            mod.set_axon_ntff_profile_hook(
                _ntff_profile_via_ctypes("/opt/axon/libaxon_pjrt.so")
            )
        except Exception:
            pass

    import concourse.bass_utils as bass_utils

    bass_utils.upload_artifacts = lambda tmpdir: tmpdir

    # walrus in this image rejects >1 sync wait per instruction: split the
    # final TileContext drain and every scheduled instruction's waits onto
    # same-engine NoOp carriers.
    import concourse.mybir as mybir
    import concourse.tile as tile
    from concourse.vector_clock import ScopedClock

    if getattr(tile.TileContext, "_wait_split_installed", False):
        return

    def _drain_and_barrier_split(self, tick_clock, wait_clock):
        import bass_rust

        d = self.nc.sync.drain()
        wait_clock.add_sem_waits(d.ins, ScopedClock({None: tick_clock.global_clock}))
        si = d.ins.sync_info
        waits = list(si.on_wait) if si is not None and si.on_wait else []
        if len(waits) > 1:
            si.on_wait = waits[:1]
            for i in range(1, len(waits)):
                d2 = self.nc.sync.drain()
                si2 = d2.ins.sync_info
                if si2 is None:
                    d2.ins.sync_info = bass_rust.SyncInfo(
                        on_wait=waits[i : i + 1], on_update=[]
                    )
                else:
                    si2.on_wait = waits[i : i + 1]
        self.nc.all_engine_barrier()
        popped = self.nc._tile_sem_poison_stack.pop()
        assert popped is self._sem_poison
        self.nc.clear_and_free_semaphores(list(self.sems.allocated().values()))
        self.nc.all_engine_barrier()

    tile.TileContext._drain_and_barrier = _drain_and_barrier_split

    _orig_commit = tile.TileContext._commit_instruction

    def _commit_split_waits(self, inst, lazy_reg_writes: bool = True):
        si = getattr(inst, "sync_info", None)
        if si is not None and si.on_wait and len(si.on_wait) > 1:
            waits = list(si.on_wait)
            engine = inst.engine
            if engine is not None and engine != mybir.EngineType.Unassigned:
                si.on_wait = waits[-1:]
                for w in waits[:-1]:
                    nop = mybir.InstNoOp(
                        name=self.nc.get_next_instruction_name(),
                        engine=engine,
                        sync_info=mybir.SyncInfo(on_wait=[w], on_update=[]),
                        bass_nofuse=True,
                    )
                    _orig_commit(self, nop, lazy_reg_writes)
        return _orig_commit(self, inst, lazy_reg_writes)

    tile.TileContext._commit_instruction = _commit_split_waits
    tile.TileContext._wait_split_installed = True


_install_shims()

import concourse.bass as bass
import concourse.mybir as mybir
import concourse.tile as tile
from concourse.bass_utils import run_bass_kernel_spmd
from concourse.masks import make_identity

f32 = mybir.dt.float32
bf16 = mybir.dt.bfloat16
P = 128
N_CORES = 8
USE_BF16 = False  # bf16 sel/attention path: ~1.1e-3 rel err, 1.25ms; fp32: 5e-6, 1.54ms


# ---------------------------------------------------------------------------
# host-side weight folding
# ---------------------------------------------------------------------------
def _blockdiag(mats):
    """mats: [H, a, b] -> blockdiag [H*a, H*b]"""
    H, a, b = mats.shape
    out = np.zeros((H * a, H * b), dtype=mats.dtype)
    for h in range(H):
        out[h * a : (h + 1) * a, h * b : (h + 1) * b] = mats[h]
    return out


def _fold_weights(qW, qb, kW, kb, vW, vb, rel_att, rel_pri, rel_msg, upd_W, upd_b):
    """Returns host-folded weight banks (see kernel layout)."""
    KN, HID, _ = qW.shape
    KR, H, DK, _ = rel_att.shape
    s = 1.0 / np.sqrt(DK)
    # l[e,h] = (q @ G_r)[h,:] . k[h,:]  with G_r = blockdiag_h(rel_att[r,h].T * pri[r,h]*s)
    Wq_fold = np.zeros((KN, KR, HID, HID), np.float32)
    qb_fold = np.zeros((KN, KR, HID), np.float32)
    Wv_fold = np.zeros((KN, KR, HID, HID), np.float32)
    vb_fold = np.zeros((KN, KR, HID), np.float32)
    for r in range(KR):
        G = _blockdiag(
            np.ascontiguousarray(rel_att[r].transpose(0, 2, 1))
            * (rel_pri[r] * s)[:, None, None]
        )
        Bm = _blockdiag(rel_msg[r])
        for t in range(KN):
            Wq_fold[t, r] = qW[t] @ G
            qb_fold[t, r] = qb[t] @ G
            Wv_fold[t, r] = vW[t] @ Bm
            vb_fold[t, r] = vb[t] @ Bm
    return dict(
        Wq_fold=Wq_fold, qb_fold=qb_fold, Wv_fold=Wv_fold, vb_fold=vb_fold,
        Wk=kW.astype(np.float32), kb=kb.astype(np.float32),
        Wu=upd_W.astype(np.float32), ub=upd_b.astype(np.float32),
    )


# ---------------------------------------------------------------------------
# host-side segmentation + stream building
# ---------------------------------------------------------------------------
def _prep(x, node_type, src, tgt, rel, CR, n_cores):
    """Segment nodes, assign edges to chunk slots, build per-core streams."""
    N, HID = x.shape
    KCOMB = 8
    C = KCOMB * CR
    cap = CR * P

    combo = (node_type[src] * 4 + rel).astype(np.int64)

    # per (node, combo) incoming-edge counts
    cnt = np.bincount(tgt * KCOMB + combo, minlength=N * KCOMB).reshape(N, KCOMB)

    # greedy segmentation over node order
    seg_start = [0]
    cur = np.zeros(KCOMB, np.int64)
    start = 0
    for n in range(N):
        c = cnt[n]
        if (n - start) >= P or np.any(cur + c > cap):
            seg_start.append(n)
            start = n
            cur = c.copy()
        else:
            cur += c
    seg_start = np.asarray(seg_start, np.int64)
    seg_end = np.append(seg_start[1:], N)
    S_total = len(seg_start)

    S = (S_total + n_cores - 1) // n_cores

    # map each node -> (segment, local index)
    node_seg = np.repeat(np.arange(S_total), seg_end - seg_start)
    node_loc = np.arange(N) - seg_start[node_seg]

    # edge -> segment, combo; slot position within (segment, combo)
    e_seg = node_seg[tgt]
    e_key = e_seg * KCOMB + combo
    order = np.argsort(e_key, kind="stable")
    ranks = np.empty(len(order), np.int64)
    keys_sorted = e_key[order]
    # rank within each (segment, combo) group
    grp_first = np.r_[0, np.flatnonzero(np.diff(keys_sorted)) + 1]
    grp_id = np.cumsum(np.r_[0, np.diff(keys_sorted) != 0])
    ranks_sorted = np.arange(len(order)) - grp_first[grp_id]
    ranks[order] = ranks_sorted
    assert ranks.max() < cap, "segment combo capacity overflow"

    e_chunk = combo * CR + ranks // P   # chunk slot 0..C-1 within segment
    e_pos = ranks % P                   # partition row within chunk
    e_core = e_seg // S
    e_sloc = e_seg % S

    # streams
    xT = np.zeros((n_cores, S, HID, C * P), np.float32)
    tgtl = np.full((n_cores, S, P, C), -1.0, np.float32)
    xsegT = np.zeros((n_cores, S, HID, 2 * P), np.float32)
    xseg = np.zeros((n_cores, S, P, HID), np.float32)
    typec = np.zeros((n_cores, S, P, 1), np.float32)
    tmask = np.zeros((n_cores, S, 2, P), np.float32)

    col = e_chunk * P + e_pos
    xT[e_core, e_sloc, :, col] = x[src]
    tgtl[e_core, e_sloc, e_pos, e_chunk] = node_loc[tgt].astype(np.float32)
    selS = np.zeros((n_cores, S, P, C * P), np.float32)
    selS[e_core, e_sloc, e_pos, e_chunk * P + node_loc[tgt]] = 1.0
    selT = np.zeros((n_cores, S, P, C * P), np.float32)
    selT[e_core, e_sloc, node_loc[tgt], e_chunk * P + e_pos] = 1.0

    n_seg = node_seg
    n_core = n_seg // S
    n_sloc = n_seg % S
    xseg[n_core, n_sloc, node_loc, :] = x
    m0 = node_type == 0
    xsegT[n_core[m0], n_sloc[m0], :, node_loc[m0]] = x[m0]
    m1 = ~m0
    xsegT[n_core[m1], n_sloc[m1], :, P + node_loc[m1]] = x[m1]
    typec[n_core, n_sloc, node_loc, 0] = node_type.astype(np.float32)
    tmask[n_core, n_sloc, node_type, node_loc] = 1.0

    unshard = (S, seg_start, seg_end)
    streams = dict(xT=xT, tgtl=tgtl, xsegT=xsegT, xseg=xseg, typec=typec,
                   tmask=tmask, sel=selS, selT=selT)
    return streams, unshard, S, C


# ---------------------------------------------------------------------------
# device program
# ---------------------------------------------------------------------------
def _build_program(S, C, CR, flags):
    """One SPMD program processing S segments of C chunk slots."""
    GROUP = 2
    NG = C // GROUP
    dt_e = bf16 if USE_BF16 else f32  # edge-path matmul operand dtype
    nc = bass.Bass("TRN2", target_bir_lowering=False, debug=False)

    xT_d = nc.declare_dram_parameter("xT", [S, P, C * P], f32, isOutput=False)
    sel_d = nc.declare_dram_parameter("sel", [S, P, C * P], f32, isOutput=False)
    selT_d = nc.declare_dram_parameter("selT", [S, P, C * P], bf16, isOutput=False)
    xsegT_d = nc.declare_dram_parameter("xsegT", [S, P, 2 * P], f32, isOutput=False)
    xseg_d = nc.declare_dram_parameter("xseg", [S, P, P], f32, isOutput=False)
    typec_d = nc.declare_dram_parameter("typec", [S, P, 1], f32, isOutput=False)
    Wkv_d = nc.declare_dram_parameter("Wkv", [P, 8 * 256], f32, isOutput=False)
    Wq_d = nc.declare_dram_parameter("Wq", [P, 8 * P], f32, isOutput=False)
    Wu_d = nc.declare_dram_parameter("Wu", [P, 2 * P], f32, isOutput=False)
    B8T_d = nc.declare_dram_parameter("B8T", [8, P], f32, isOutput=False)
    if flags["kv_bias"]:
        bkv_d = nc.declare_dram_parameter("bkv", [1, 8 * 256], f32, isOutput=False)
    if flags["q_bias"]:
        bq_d = nc.declare_dram_parameter("bq", [1, 8 * P], f32, isOutput=False)
        tmask_d = nc.declare_dram_parameter("tmask", [S, 2, P], f32, isOutput=False)
    if flags["u_bias"]:
        bu_d = nc.declare_dram_parameter("bu", [1, 2 * P], f32, isOutput=False)
    if flags["ln_affine"]:
        lns_d = nc.declare_dram_parameter("lns", [1, 2 * P], f32, isOutput=False)
        lnb_d = nc.declare_dram_parameter("lnb", [1, 2 * P], f32, isOutput=False)
    out_d = nc.declare_dram_parameter("out", [S, P, P], f32, isOutput=True)

    from contextlib import ExitStack

    with tile.TileContext(nc) as tc, ExitStack() as ctx:
        cpool = ctx.enter_context(tc.tile_pool(name="const", bufs=1))
        ones = cpool.tile([1, P], f32)
        nc.gpsimd.memset(ones[:], 1.0)
        eps_t = cpool.tile([P, 1], f32)
        nc.gpsimd.memset(eps_t[:], 1e-5)

        Wkv_t = cpool.tile([P, 8 * 256], f32)
        nc.sync.dma_start(Wkv_t[:], Wkv_d[:])
        Wq_t = cpool.tile([P, 8 * P], f32)
        nc.sync.dma_start(Wq_t[:], Wq_d[:])
        Wu_t = cpool.tile([P, 2 * P], f32)
        nc.sync.dma_start(Wu_t[:], Wu_d[:])
        B8T_t = cpool.tile([8, P], f32)
        nc.sync.dma_start(B8T_t[:], B8T_d[:])
        if flags["kv_bias"]:
            bkv_t = cpool.tile([1, 8 * 256], f32)
            nc.sync.dma_start(bkv_t[:], bkv_d[:])
        if flags["q_bias"]:
            bq_t = cpool.tile([1, 8 * P], f32)
            nc.sync.dma_start(bq_t[:], bq_d[:])
        if flags["u_bias"]:
            bu_t = cpool.tile([1, 2 * P], f32)
            nc.sync.dma_start(bu_t[:], bu_d[:])
        if flags["ln_affine"]:
            lns_t = cpool.tile([P, 2 * P], f32)
            nc.sync.dma_start(lns_t[:], lns_d[:1, :].to_broadcast([P, 2 * P]))
            lnb_t = cpool.tile([P, 2 * P], f32)
            nc.sync.dma_start(lnb_t[:], lnb_d[:1, :].to_broadcast([P, 2 * P]))

        # stream pools
        xT_p = ctx.enter_context(tc.tile_pool(name="xT", bufs=3))
        tg_p = ctx.enter_context(tc.tile_pool(name="tg", bufs=2))
        xsT_p = ctx.enter_context(tc.tile_pool(name="xsT", bufs=2))
        xs_p = ctx.enter_context(tc.tile_pool(name="xs", bufs=2))
        ty_p = ctx.enter_context(tc.tile_pool(name="ty", bufs=2))
        sel_p = ctx.enter_context(tc.tile_pool(name="sel", bufs=3))
        sb_p = ctx.enter_context(tc.tile_pool(name="sb", bufs=4))      # slab SBUF
        node_p = ctx.enter_context(tc.tile_pool(name="node", bufs=2))  # node-phase

        kv_ps = ctx.enter_context(tc.tile_pool(name="kvps", bufs=2, space="PSUM"))
        st_ps = ctx.enter_context(tc.tile_pool(name="stps", bufs=1, space="PSUM"))
        qe_ps = ctx.enter_context(tc.tile_pool(name="qeps", bufs=2, space="PSUM"))
        u_ps = ctx.enter_context(tc.tile_pool(name="ups", bufs=1, space="PSUM"))
        d_ps = ctx.enter_context(tc.tile_pool(name="dps", bufs=1, space="PSUM"))
        n_ps = ctx.enter_context(tc.tile_pool(name="nps", bufs=1, space="PSUM"))

        for s in range(S):
            # ---- loads ----
            xT_t = xT_p.tile([P, C * P], f32)
            nc.sync.dma_start(xT_t[:], xT_d[s])
            sel_t = sel_p.tile([P, C * P], f32)
            nc.sync.dma_start(sel_t[:], sel_d[s])
            selT_t = sel_p.tile([P, C * P], bf16, tag="selT")
            nc.sync.dma_start(selT_t[:], selT_d[s])
            xsT_t = xsT_p.tile([P, 2 * P], f32)
            nc.sync.dma_start(xsT_t[:], xsegT_d[s])
            xs_t = xs_p.tile([P, P], f32)
            nc.sync.dma_start(xs_t[:], xseg_d[s])
            ty_t = ty_p.tile([P, 1], f32)
            nc.sync.dma_start(ty_t[:], typec_d[s])

            # ---- q_rel banks: q folded per relation ----
            qr_ps = st_ps.tile([P, 4 * P], f32, tag="st")
            for r in range(4):
                nc.tensor.matmul(
                    qr_ps[:, r * P : (r + 1) * P],
                    lhsT=xsT_t[:, 0:P],
                    rhs=Wq_t[:, r * P : (r + 1) * P],
                    start=True, stop=False,
                )
                nc.tensor.matmul(
                    qr_ps[:, r * P : (r + 1) * P],
                    lhsT=xsT_t[:, P : 2 * P],
                    rhs=Wq_t[:, (4 + r) * P : (5 + r) * P],
                    start=False, stop=not flags["q_bias"],
                )
            if flags["q_bias"]:
                # typed bias via rank-1 mms with host-streamed per-type masks
                tm_t = node_p.tile([2, P], f32, tag="tm")
                nc.sync.dma_start(tm_t[:], tmask_d[s])
                for r in range(4):
                    nc.tensor.matmul(
                        qr_ps[:, r * P : (r + 1) * P],
                        lhsT=tm_t[0:1, :], rhs=bq_t[:1, r * P : (r + 1) * P],
                        start=False, stop=False,
                    )
                    nc.tensor.matmul(
                        qr_ps[:, r * P : (r + 1) * P],
                        lhsT=tm_t[1:2, :], rhs=bq_t[:1, (4 + r) * P : (5 + r) * P],
                        start=False, stop=True,
                    )
            qr_sb = sb_p.tile([P, 4 * P], bf16, tag="qr")
            nc.scalar.copy(qr_sb[:], qr_ps[:])
            qr_lo = sb_p.tile([P, 4 * P], bf16, tag="qrlo")
            nc.vector.tensor_tensor(
                out=qr_lo[:], in0=qr_ps[:], in1=qr_sb[:],
                op=mybir.AluOpType.subtract,
            )

            # ---- accumulators (zeroed by the c==0 scatter matmuls) ----
            U_t = u_ps.tile([P, P], f32, tag="U")
            den_t = d_ps.tile([8, P], f32, tag="den")

            # ---- edge chunk groups ----
            for g in range(NG):
                kv_t = kv_ps.tile([P, GROUP * 256], f32, tag="kv")
                for j in range(GROUP):
                    c = g * GROUP + j
                    m = c // CR  # combo = t*4+r
                    nc.tensor.matmul(
                        kv_t[:, j * 256 : (j + 1) * 256],
                        lhsT=xT_t[:, c * P : (c + 1) * P],
                        rhs=Wkv_t[:, m * 256 : (m + 1) * 256],
                        start=True, stop=not flags["kv_bias"],
                    )
                    if flags["kv_bias"]:
                        nc.tensor.matmul(
                            kv_t[:, j * 256 : (j + 1) * 256],
                            lhsT=ones[:1, :],
                            rhs=bkv_t[:1, m * 256 : (m + 1) * 256],
                            start=False, stop=True,
                        )

                qe_t = qe_ps.tile([P, GROUP * P], f32, tag="qe")
                for j in range(GROUP):
                    c = g * GROUP + j
                    r = (c // CR) % 4
                    nc.tensor.matmul(
                        qe_t[:, j * P : (j + 1) * P],
                        lhsT=selT_t[:, c * P : (c + 1) * P],
                        rhs=qr_sb[:, r * P : (r + 1) * P],
                        start=True, stop=False,
                    )
                    nc.tensor.matmul(
                        qe_t[:, j * P : (j + 1) * P],
                        lhsT=selT_t[:, c * P : (c + 1) * P],
                        rhs=qr_lo[:, r * P : (r + 1) * P],
                        start=False, stop=True,
                    )
                qe_sb = sb_p.tile([P, GROUP * P], f32, tag="qes")
                nc.scalar.copy(qe_sb[:], qe_t[:])

                kvv = kv_t[:].rearrange("p (j s) -> p j s", s=256)
                qk_sb = sb_p.tile([P, GROUP * P], f32, tag="qk")
                nc.vector.tensor_tensor(
                    out=qk_sb[:].rearrange("p (j n) -> p j n", j=GROUP),
                    in0=qe_sb[:].rearrange("p (j n) -> p j n", j=GROUP),
                    in1=kvv[:, :, 0:P],
                    op=mybir.AluOpType.mult,
                )
                l_sb = sb_p.tile([P, GROUP * 8], f32, tag="l")
                nc.vector.reduce_sum(
                    out=l_sb[:],
                    in_=qk_sb[:].rearrange("p (a k) -> p a k", k=16),
                    axis=mybir.AxisListType.X,
                )
                w_sb = sb_p.tile([P, GROUP * 8], dt_e, tag="w")
                nc.scalar.activation(w_sb[:], l_sb[:], mybir.ActivationFunctionType.Exp)
                wv_sb = sb_p.tile([P, GROUP * P], dt_e, tag="wv")
                nc.vector.tensor_tensor(
                    out=wv_sb[:].rearrange("p (j h k) -> p j h k", j=GROUP, h=8),
                    in0=kvv[:, :, P:256].rearrange("p j (h k) -> p j h k", h=8),
                    in1=w_sb[:].rearrange("p (j h) -> p j h", j=GROUP)[:, :, :, None]
                    .to_broadcast([P, GROUP, 8, 16]),
                    op=mybir.AluOpType.mult,
                )
                last = (g == NG - 1)
                for j in range(GROUP):
                    c = g * GROUP + j
                    nc.tensor.matmul(
                        U_t[:],
                        lhsT=wv_sb[:, j * P : (j + 1) * P],
                        rhs=sel_t[:, c * P : (c + 1) * P],
                        start=c == 0, stop=last and j == GROUP - 1,
                    )
                    nc.tensor.matmul(
                        den_t[:],
                        lhsT=w_sb[:, j * 8 : (j + 1) * 8],
                        rhs=sel_t[:, c * P : (c + 1) * P],
                        start=c == 0, stop=last and j == GROUP - 1,
                    )

            # ---- segment epilogue ----
            dmax = node_p.tile([8, P], f32, tag="dmax")
            nc.vector.tensor_scalar(
                out=dmax[:], in0=den_t[:], scalar1=1e-9, scalar2=None,
                op0=mybir.AluOpType.max,
            )
            drec = node_p.tile([8, P], f32, tag="drec")
            nc.vector.reciprocal(drec[:], dmax[:])
            dbc_t = n_ps.tile([P, P], f32, tag="nps")
            nc.tensor.matmul(dbc_t[:], lhsT=B8T_t[:], rhs=drec[:], start=True, stop=True)
            dbc_sb = node_p.tile([P, P], f32, tag="dbc")
            nc.scalar.copy(dbc_sb[:], dbc_t[:])
            aggrn = node_p.tile([P, P], f32, tag="aggrn")
            nc.vector.tensor_tensor(
                out=aggrn[:], in0=U_t[:], in1=dbc_sb[:], op=mybir.AluOpType.mult
            )
            upd_t = n_ps.tile([P, 2 * P], f32, tag="nps")
            for t in range(2):
                nc.tensor.matmul(
                    upd_t[:, t * P : (t + 1) * P],
                    lhsT=aggrn[:], rhs=Wu_t[:, t * P : (t + 1) * P],
                    start=True, stop=not flags["u_bias"],
                )
                if flags["u_bias"]:
                    nc.tensor.matmul(
                        upd_t[:, t * P : (t + 1) * P],
                        lhsT=ones[:1, :], rhs=bu_t[:1, t * P : (t + 1) * P],
                        start=False, stop=True,
                    )
            upd_sb = node_p.tile([P, 2 * P], f32, tag="upd")
            nc.scalar.copy(upd_sb[:], upd_t[:])
            diff = node_p.tile([P, P], f32, tag="diff")
            nc.vector.tensor_tensor(
                out=diff[:], in0=upd_sb[:, P : 2 * P], in1=upd_sb[:, 0:P],
                op=mybir.AluOpType.subtract,
            )
            # blended update + residual: upd0 + typec*diff + x
            b1 = node_p.tile([P, P], f32, tag="b1")
            nc.vector.scalar_tensor_tensor(
                out=b1[:], in0=diff[:], scalar=ty_t[:, 0:1], in1=upd_sb[:, 0:P],
                op0=mybir.AluOpType.mult, op1=mybir.AluOpType.add,
            )
            h_t = node_p.tile([P, P], f32, tag="h")
            nc.vector.tensor_tensor(
                out=h_t[:], in0=b1[:], in1=xs_t[:], op=mybir.AluOpType.add
            )
            # layernorm
            musum = node_p.tile([P, 1], f32, tag="musum")
            nc.vector.reduce_sum(out=musum[:], in_=h_t[:], axis=mybir.AxisListType.X)
            mean = node_p.tile([P, 1], f32, tag="mean")
            nc.scalar.mul(mean[:], musum[:], 1.0 / P)
            xc = node_p.tile([P, P], f32, tag="xc")
            nc.vector.tensor_scalar(
                out=xc[:], in0=h_t[:], scalar1=mean[:, 0:1], scalar2=None,
                op0=mybir.AluOpType.subtract,
            )
            sq = node_p.tile([P, P], f32, tag="sq")
            ss = node_p.tile([P, 1], f32, tag="ss")
            nc.scalar.activation(
                sq[:], xc[:], mybir.ActivationFunctionType.Square, accum_out=ss[:]
            )
            sd = node_p.tile([P, 1], f32, tag="sd")
            nc.scalar.activation(
                sd[:], ss[:], mybir.ActivationFunctionType.Sqrt,
                bias=eps_t[:, 0:1], scale=1.0 / P,
            )
            rstd = node_p.tile([P, 1], f32, tag="rstd")
            nc.vector.reciprocal(rstd[:], sd[:])
            o_t = node_p.tile([P, P], f32, tag="o")
            nc.vector.tensor_scalar(
                out=o_t[:], in0=xc[:], scalar1=rstd[:, 0:1], scalar2=None,
                op0=mybir.AluOpType.mult,
            )
            if flags["ln_affine"]:
                sdiff = node_p.tile([P, P], f32, tag="sdiff")
                nc.vector.tensor_tensor(
                    out=sdiff[:], in0=lns_t[:, P : 2 * P], in1=lns_t[:, 0:P],
                    op=mybir.AluOpType.subtract,
                )
                seff = node_p.tile([P, P], f32, tag="seff")
                nc.vector.scalar_tensor_tensor(
                    out=seff[:], in0=sdiff[:], scalar=ty_t[:, 0:1], in1=lns_t[:, 0:P],
                    op0=mybir.AluOpType.mult, op1=mybir.AluOpType.add,
                )
                o2 = node_p.tile([P, P], f32, tag="o2")
                nc.vector.tensor_tensor(
                    out=o2[:], in0=o_t[:], in1=seff[:], op=mybir.AluOpType.mult
                )
                bdiff = node_p.tile([P, P], f32, tag="bdiff")
                nc.vector.tensor_tensor(
                    out=bdiff[:], in0=lnb_t[:, P : 2 * P], in1=lnb_t[:, 0:P],
                    op=mybir.AluOpType.subtract,
                )
                beff = node_p.tile([P, P], f32, tag="beff")
                nc.vector.scalar_tensor_tensor(
                    out=beff[:], in0=bdiff[:], scalar=ty_t[:, 0:1], in1=lnb_t[:, 0:P],
                    op0=mybir.AluOpType.mult, op1=mybir.AluOpType.add,
                )
                o3 = node_p.tile([P, P], f32, tag="o3")
                nc.vector.tensor_tensor(
                    out=o3[:], in0=o2[:], in1=beff[:], op=mybir.AluOpType.add
                )
                o_t = o3
            nc.sync.dma_start(out_d[s], o_t[:])

    return nc


# ---------------------------------------------------------------------------
# entry point
# ---------------------------------------------------------------------------
def kernel(x, node_type, edge_index, edge_rel, qW, qb, kW, kb, vW, vb,
           rel_att, rel_pri, rel_msg, upd_W, upd_b, ln_scale, ln_bias,
           CR=2, trace=False):
    x = np.asarray(x, np.float32)
    node_type = np.asarray(node_type).astype(np.int64)
    edge_index = np.asarray(edge_index).astype(np.int64)
    edge_rel = np.asarray(edge_rel).astype(np.int64)
    qW, qb, kW, kb = (np.asarray(a, np.float32) for a in (qW, qb, kW, kb))
    vW, vb = np.asarray(vW, np.float32), np.asarray(vb, np.float32)
    rel_att = np.asarray(rel_att, np.float32)
    rel_pri = np.asarray(rel_pri, np.float32)
    rel_msg = np.asarray(rel_msg, np.float32)
    upd_W, upd_b = np.asarray(upd_W, np.float32), np.asarray(upd_b, np.float32)
    ln_scale, ln_bias = np.asarray(ln_scale, np.float32), np.asarray(ln_bias, np.float32)

    N, HID = x.shape
    src, tgt = edge_index[0], edge_index[1]

    fw = _fold_weights(qW, qb, kW, kb, vW, vb, rel_att, rel_pri, rel_msg, upd_W, upd_b)

    flags = dict(
        kv_bias=bool(np.any(fw["kb"]) or np.any(fw["vb_fold"])),
        q_bias=bool(np.any(fw["qb_fold"])),
        u_bias=bool(np.any(fw["ub"])),
        ln_affine=bool(np.any(ln_scale != 1.0) or np.any(ln_bias != 0.0)),
    )

    streams, unshard, S, C = _prep(x, node_type, src, tgt, edge_rel, CR, N_CORES)

    # weight banks
    Wkv = np.zeros((P, 8 * 256), np.float32)
    bkv = np.zeros((1, 8 * 256), np.float32)
    Wq = np.zeros((P, 8 * P), np.float32)
    bq = np.zeros((1, 8 * P), np.float32)
    for t in range(2):
        for r in range(4):
            m = t * 4 + r
            Wkv[:, m * 256 : m * 256 + P] = fw["Wk"][t]
            Wkv[:, m * 256 + P : (m + 1) * 256] = fw["Wv_fold"][t, r]
            bkv[0, m * 256 : m * 256 + P] = fw["kb"][t]
            bkv[0, m * 256 + P : (m + 1) * 256] = fw["vb_fold"][t, r]
            Wq[:, m * P : (m + 1) * P] = fw["Wq_fold"][t, r]
            bq[0, m * P : (m + 1) * P] = fw["qb_fold"][t, r]
    Wu = np.concatenate([fw["Wu"][0], fw["Wu"][1]], axis=1)
    bu = np.concatenate([fw["ub"][0], fw["ub"][1]])[None, :]
    B8T = (np.arange(P)[None, :] // 16 == np.arange(8)[:, None]).astype(np.float32)

    nc = _build_program(S, C, CR, flags)

    import ml_dtypes

    streams["selT"] = streams["selT"].astype(ml_dtypes.bfloat16)

    in_maps = []
    for c in range(N_CORES):
        m = {
            "xT": streams["xT"][c],
            "sel": streams["sel"][c], "selT": streams["selT"][c],
            "xsegT": streams["xsegT"][c], "xseg": streams["xseg"][c],
            "typec": streams["typec"][c],
            "Wkv": Wkv, "Wq": Wq, "Wu": Wu, "B8T": B8T,
        }
        if flags["kv_bias"]:
            m["bkv"] = bkv
        if flags["q_bias"]:
            m["bq"] = bq
            m["tmask"] = streams["tmask"][c]
        if flags["u_bias"]:
            m["bu"] = bu
        if flags["ln_affine"]:
            m["lns"] = np.concatenate([ln_scale[0], ln_scale[1]])[None, :]
            m["lnb"] = np.concatenate([ln_bias[0], ln_bias[1]])[None, :]
        in_maps.append(m)

    res = run_bass_kernel_spmd(nc, in_maps, list(range(N_CORES)), trace=trace)

    S_, seg_start, seg_end = unshard
    out = np.empty((N, HID), np.float32)
    for i in range(len(seg_start)):
        c, sl = i // S_, i % S_
        a, b = seg_start[i], seg_end[i]
        out[a:b] = res.results[c]["out"][sl, : b - a]
    kernel.last_exec_time_ns = res.exec_time_ns
    kernel.last_results = res
    return out


kernel.last_exec_time_ns = None
kernel.last_results = None
